# revision 7
# baseline (speedup 1.0000x reference)
"""Trainium2 Bass kernel for the ModelB graph loss.

Strategy (v2): exploit that only the tiny ARI branch (n<=50 batches) needs
per-batch sums; every other reduction in the loss is GLOBAL across batches.

  * Dense p-data (all valid [n,n] elements of every batch) is packed
    free-form (zero padding waste) and split evenly across the 8 cores.
    The two dense log-sums use a product-pairing identity
    ln(a)+ln(b)+ln(c)+ln(d) = ln(a*b*c*d): two cheap DVE multiply levels
    reduce the ACT Ln element count 4x.  The hi half of the tile stores
    (x-1) so the (1-p) chain needs no extra "1-x" pass:
       (x_hi')+1)*x_lo = p_lo*p_hi     (x_lo-1)*(x_hi') = q_lo*q_hi
  * adjacency is binary, so  sum(a*dlt) = sum over edge-gathered p of
    logit, and sum((r-a)^2) = sum(r^2) - 2*sum_e(r) + n_edges.  The host
    gathers edge positions (pure selection); dense `a` never ships.
  * sum(r^2) runs on the idle TensorEngine: accumulate chunk products
    R_c^T R_c into one PSUM bank; the diagonal then holds per-column-class
    totals, extracted with one tensor_tensor_reduce against an identity.
  * ARI batches (0.7% of elements) go in a small partition-disjoint tile;
    per-batch sums come out of per-partition accumulators.
  * Host does layout/gather/dtype packing and the final scalar algebra in
    float64; all O(N^2) float reductions happen on device.
"""

import sys

for _p in ("/opt/trn_rl_repo", "/root/.axon_site/_ro/trn_rl_repo"):
    if _p not in sys.path:
        sys.path.insert(0, _p)

from contextlib import ExitStack

import numpy as np

import concourse.bass as bass  # noqa: F401  (registers engine methods)
import concourse.tile as tile
from concourse import bacc, mybir
from concourse.bass_utils import run_bass_kernel_spmd

N_CORES = 8
B, N, C = 64, 512, 2
EPS = 1e-8

_FT = mybir.dt.float32
_BF = mybir.dt.bfloat16
_F8 = mybir.dt.float8e4
_AF = mybir.ActivationFunctionType
_OP = mybir.AluOpType

import ml_dtypes

_BF_NP = ml_dtypes.bfloat16
_F8_NP = ml_dtypes.float8_e4m3fn

# ---- SPMD-uniform geometry (derived from B=64, N=512 worst cases) ----
NCH = 2            # xp pipeline chunks
DW = 494           # dense cols per quarter-slot per chunk
EW = 50            # edge cols per quarter-slot per chunk
HW = 2 * (DW + EW)         # half-width per chunk  (1088)
XPW = 2 * HW               # xp chunk width        (2176)
DENSE_QUADS = NCH * DW * 128   # dense quad positions per core (126464)
EDGE_QUADS = NCH * EW * 128    # edge quad positions per core  (12800)
RW = 31 * 128      # xr cols (3968)
REW = 400          # edge-r cols
FS = 144           # small-tile cols
AUXW = REW + 3 * FS  # aux width (832): [re | ps | as | sel]
CW = 24            # coord cols per tensor
NS = 13            # stats cols

_build_cache: dict = {}


def _build():
    nc = bacc.Bacc("TRN2", target_bir_lowering=False, debug=False,
                   num_devices=N_CORES)

    xp_in = [nc.dram_tensor(f"xp{i}", [128, XPW], _BF,
                            kind="ExternalInput").ap() for i in range(NCH)]
    xr_in = nc.dram_tensor("xr", [128, RW], _F8, kind="ExternalInput").ap()
    aux_in = nc.dram_tensor("aux", [128, AUXW], _BF,
                            kind="ExternalInput").ap()
    crd_in = nc.dram_tensor("crd", [128, 2 * CW], _FT,
                            kind="ExternalInput").ap()
    idn_in = nc.dram_tensor("idn", [128, 128], _FT,
                            kind="ExternalInput").ap()
    st_out = nc.dram_tensor("st", [128, NS], _FT,
                            kind="ExternalOutput").ap()

    with tile.TileContext(nc) as tc, ExitStack() as ctx:
        pin = ctx.enter_context(tc.tile_pool(name="pin", bufs=NCH))
        pmid = ctx.enter_context(tc.tile_pool(name="pmid", bufs=2 * NCH))
        pscr = ctx.enter_context(tc.tile_pool(name="pscr", bufs=4))
        psml = ctx.enter_context(tc.tile_pool(name="psml", bufs=1))
        pstat = ctx.enter_context(tc.tile_pool(name="pstat", bufs=1))
        ppsum = ctx.enter_context(tc.tile_pool(name="ppsum", bufs=1,
                                               space="PSUM"))

        st = pstat.tile([128, NS], _FT, tag="st")

        def stc(i):
            return st[:, i:i + 1]

        # ---- input DMAs ----
        txp = []
        for i in range(NCH):
            t = pin.tile([128, XPW], _BF, tag="txp")
            nc.sync.dma_start(t[:], xp_in[i][:])
            txp.append(t)
        txr = psml.tile([128, RW], _F8, tag="txr")
        nc.gpsimd.dma_start(txr[:], xr_in[:])
        taux = psml.tile([128, AUXW], _BF, tag="taux")
        nc.gpsimd.dma_start(taux[:], aux_in[:])
        tcrd = psml.tile([128, 2 * CW], _FT, tag="tcrd")
        nc.gpsimd.dma_start(tcrd[:], crd_in[:])
        tidn = psml.tile([128, 128], _FT, tag="tidn")
        nc.gpsimd.dma_start(tidn[:], idn_in[:])

        zb = psml.tile([128, REW], _BF, tag="zb")
        nc.gpsimd.memset(zb[:], 0.0)
        zf = psml.tile([128, CW], _FT, tag="zf")
        nc.gpsimd.memset(zf[:], 0.0)

        # ---- PE: sum(r^2) via accumulated R^T R diagonal ----
        psum = ppsum.tile([128, 128], _FT, tag="psum")
        for c in range(31):
            nc.tensor.matmul(psum[:], txr[:, 128 * c:128 * (c + 1)],
                             txr[:, 128 * c:128 * (c + 1)],
                             start=(c == 0), stop=(c == 30))

        # ---- dense+edge paired log chains, per chunk ----
        for i in range(NCH):
            x = txp[i]
            lo = x[:, 0:HW]
            hi = x[:, HW:XPW]
            p2 = pmid.tile([128, HW], _BF, tag="p2")
            # (x_hi' + 1) * x_lo  =  p_lo * p_hi
            nc.vector.scalar_tensor_tensor(p2[:], hi, 1.0, lo,
                                           _OP.add, _OP.mult)
            q2 = pmid.tile([128, HW], _BF, tag="q2")
            # (x_lo - 1) * x_hi'  =  q_lo * q_hi
            nc.vector.scalar_tensor_tensor(q2[:], lo, 1.0, hi,
                                           _OP.subtract, _OP.mult)
            p4 = pmid.tile([128, HW // 2], _BF, tag="p4")
            nc.vector.tensor_mul(p4[:], p2[:, 0:HW // 2], p2[:, HW // 2:HW])
            q4 = pmid.tile([128, HW // 2], _BF, tag="q4")
            nc.vector.tensor_mul(q4[:], q2[:, 0:HW // 2], q2[:, HW // 2:HW])

            lp4 = pscr.tile([128, HW // 2], _BF, tag="lp4")
            nc.scalar.activation(lp4[:], p4[:], _AF.Ln)
            lq4 = pscr.tile([128, HW // 2], _BF, tag="lq4")
            nc.scalar.activation(lq4[:], q4[:], _AF.Ln)

            dcmb = pscr.tile([128, DW], _BF, tag="dcmb")
            # (lp4/19 + lq4); host multiplies by 0.95
            nc.vector.scalar_tensor_tensor(
                dcmb[:], lp4[:, 0:DW], 1.0 / 19.0, lq4[:, 0:DW],
                _OP.mult, _OP.add, accum_out=stc(0 + i))
            ecmb = pscr.tile([128, EW], _BF, tag="ecmb")
            nc.vector.scalar_tensor_tensor(
                ecmb[:], lp4[:, DW:DW + EW], 1.0, lq4[:, DW:DW + EW],
                _OP.mult, _OP.subtract, accum_out=stc(2 + i))

        # ---- small (ARI) tile: per-batch stats by partition range ----
        ps_ = taux[:, REW:REW + FS]
        as_ = taux[:, REW + FS:REW + 2 * FS]
        sel_ = taux[:, REW + 2 * FS:REW + 3 * FS]
        lps = psml.tile([128, FS], _BF, tag="lps")
        nc.scalar.activation(lps[:], ps_, _AF.Ln)
        lqs = psml.tile([128, FS], _BF, tag="lqs")
        nc.scalar.activation(lqs[:], ps_, _AF.Ln, bias=1.0, scale=-1.0)
        dlt = psml.tile([128, FS], _BF, tag="dlt")
        nc.gpsimd.tensor_sub(dlt[:], lps[:], lqs[:])

        sscr = psml.tile([128, FS], _BF, tag="sscr")
        nc.vector.scalar_tensor_tensor(sscr[:], lqs[:], 1.0, sel_,
                                       _OP.mult, _OP.mult,
                                       accum_out=stc(8))
        spd = psml.tile([128, FS], _BF, tag="spd")
        nc.vector.scalar_tensor_tensor(spd[:], ps_, 1.0, dlt[:],
                                       _OP.mult, _OP.mult,
                                       accum_out=stc(9))
        spa = psml.tile([128, FS], _BF, tag="spa")
        nc.vector.scalar_tensor_tensor(spa[:], ps_, 1.0, as_,
                                       _OP.mult, _OP.mult,
                                       accum_out=stc(10))
        sp2 = psml.tile([128, FS], _BF, tag="sp2")
        nc.vector.scalar_tensor_tensor(sp2[:], ps_, 1.0, ps_,
                                       _OP.mult, _OP.mult,
                                       accum_out=stc(11))
        sh = psml.tile([128, FS], _BF, tag="sh")
        nc.vector.scalar_tensor_tensor(sh[:], ps_, 0.5, zb[:, 0:FS],
                                       _OP.subtract, _OP.add)
        sab = psml.tile([128, FS], _BF, tag="sab")
        nc.vector.scalar_tensor_tensor(sab[:], sh[:], -1.0, sh[:],
                                       _OP.mult, _OP.max,
                                       accum_out=stc(12))

        # ---- edge-r sum ----
        rscr = psml.tile([128, REW], _BF, tag="rscr")
        nc.vector.scalar_tensor_tensor(rscr[:], taux[:, 0:REW], 1.0,
                                       zb[:], _OP.mult, _OP.add,
                                       accum_out=stc(5))

        # ---- coords: d = pc - pt; huber via relu split ----
        d = psml.tile([128, CW], _FT, tag="d")
        nc.gpsimd.tensor_sub(d[:], tcrd[:, 0:CW], tcrd[:, CW:2 * CW])
        ad = psml.tile([128, CW], _FT, tag="ad")
        nc.vector.scalar_tensor_tensor(ad[:], d[:], -1.0, d[:],
                                       _OP.mult, _OP.max)
        hb = psml.tile([128, CW], _FT, tag="hb")
        nc.vector.scalar_tensor_tensor(hb[:], ad[:], -1.0, zf[:],
                                       _OP.add, _OP.max)
        d2 = psml.tile([128, CW], _FT, tag="d2")
        nc.vector.scalar_tensor_tensor(d2[:], d[:], 1.0, d[:],
                                       _OP.mult, _OP.mult,
                                       accum_out=stc(6))
        h2 = psml.tile([128, CW], _FT, tag="h2")
        nc.vector.scalar_tensor_tensor(h2[:], hb[:], 1.0, hb[:],
                                       _OP.mult, _OP.mult,
                                       accum_out=stc(7))

        # ---- PSUM diagonal extract ----
        xscr = psml.tile([128, 128], _FT, tag="xscr")
        nc.vector.scalar_tensor_tensor(xscr[:], psum[:], 1.0, tidn[:],
                                       _OP.mult, _OP.mult,
                                       accum_out=stc(4))

        nc.sync.dma_start(st_out[:], st[:])

    nc.compile()
    return nc


def _pack_quads(stream, n_quads_per_chunk, n_chunks, pad_lo, pad_hi):
    """stream (f32) -> per-chunk (lo_dA, hi_dA, lo_dB, hi_dB) bf16 planes.

    Quad k holds stream[4k:4k+4]; hi planes store (x-1).
    Returns list of (loA, hiA, loB, hiB) f32 arrays [128, width] per chunk.
    """
    cap = 4 * n_quads_per_chunk * n_chunks
    s = np.full(cap, np.nan, np.float32)
    s[:len(stream)] = stream
    q = s.reshape(-1, 4)  # [quads, 4]
    out = []
    for c in range(n_chunks):
        qc = q[c * n_quads_per_chunk:(c + 1) * n_quads_per_chunk]
        w = n_quads_per_chunk // 128
        loA = qc[:, 0].reshape(128, w).copy()
        hiA = qc[:, 1].reshape(128, w) - 1.0
        loB = qc[:, 2].reshape(128, w).copy()
        hiB = qc[:, 3].reshape(128, w) - 1.0
        for a, pv in ((loA, pad_lo), (loB, pad_lo)):
            a[np.isnan(a)] = pv
        for a, pv in ((hiA, pad_hi - 1.0), (hiB, pad_hi - 1.0)):
            a[np.isnan(a)] = pv
        out.append((loA, hiA, loB, hiB))
    return out


def _even_split(arr, k):
    """Split 1-D arr into k nearly equal spans."""
    bounds = np.linspace(0, len(arr), k + 1).astype(np.int64)
    return [arr[bounds[i]:bounds[i + 1]] for i in range(k)]


def _huber(x):
    ax = np.abs(x)
    return np.where(ax <= 1.0, 0.5 * x * x, ax - 0.5)


def kernel(predicted_coords, adjacency_matrix, node_counts, raw_similarity,
           temperature, residual_weight, points, adjacency, node_masks,
           _want_results=None):
    masks = np.asarray(node_masks).astype(bool)
    n_list = masks.sum(axis=1).astype(np.int64)

    if "nc" not in _build_cache:
        _build_cache["nc"] = _build()
    nc = _build_cache["nc"]

    p_full = np.asarray(adjacency_matrix, dtype=np.float32)
    a_full = np.asarray(adjacency, dtype=np.float32)
    r_full = np.asarray(raw_similarity, dtype=np.float32)
    pc_full = np.ascontiguousarray(predicted_coords, dtype=np.float32)
    pt_full = np.ascontiguousarray(points, dtype=np.float32)

    # ---- host-side selection / packing (no float math on data) ----
    valid = []
    for b in range(B):
        n = int(n_list[b])
        valid.append(None if masks[b, :n].all() else np.flatnonzero(masks[b]))

    p_blocks, r_blocks, pe_blocks, re_blocks = [], [], [], []
    ec_list = []          # edge count per batch
    sgn_cnt = []          # (unused placeholder)
    pcv, ptv = [], []
    for b in range(B):
        n = int(n_list[b])
        if valid[b] is None:
            pb = p_full[b, :n, :n]
            ab = a_full[b, :n, :n]
            rb = r_full[b, :n, :n]
            pcb = pc_full[b, :n]
            ptb = pt_full[b, :n]
        else:
            ix = np.ix_(valid[b], valid[b])
            pb = p_full[b][ix]
            ab = a_full[b][ix]
            rb = r_full[b][ix]
            pcb = pc_full[b][valid[b]]
            ptb = pt_full[b][valid[b]]
        e = ab > 0.5
        p_blocks.append(pb.ravel())
        r_blocks.append(rb.ravel())
        pe_blocks.append(pb[e])
        re_blocks.append(rb[e])
        ec_list.append(int(e.sum()))
        pcv.append(pcb.ravel())
        ptv.append(ptb.ravel())

    dense_p = np.concatenate(p_blocks)
    dense_r = np.concatenate(r_blocks)
    edge_p = np.concatenate(pe_blocks)
    edge_r = np.concatenate(re_blocks)
    pc_s = np.concatenate(pcv)
    pt_s = np.concatenate(ptv)
    e_tot = float(sum(ec_list))

    dense_p_sp = _even_split(dense_p, N_CORES)
    dense_r_sp = _even_split(dense_r, N_CORES)
    edge_p_sp = _even_split(edge_p, N_CORES)
    edge_r_sp = _even_split(edge_r, N_CORES)
    pc_sp = _even_split(pc_s, N_CORES)
    pt_sp = _even_split(pt_s, N_CORES)

    # small (ARI) batches: first-fit-decreasing into 8 cores, cap 128 rows
    small = [b for b in range(B) if n_list[b] <= 50]
    order = sorted(small, key=lambda b: -n_list[b])
    bins = [[] for _ in range(N_CORES)]
    fill = [0] * N_CORES
    for b in order:
        k = min(range(N_CORES), key=lambda i: fill[i])
        assert fill[k] + n_list[b] <= 128, "small batches don't fit"
        bins[k].append(b)
        fill[k] += int(n_list[b])

    ident = np.eye(128, dtype=np.float32)

    in_maps = []
    small_layout = []
    for c in range(N_CORES):
        im = {}
        dp = dense_p_sp[c]
        ep = edge_p_sp[c]
        dq = _pack_quads(dp, DW * 128, NCH, 0.5, 0.5)
        eq = _pack_quads(ep, EW * 128, NCH, 0.5, 0.5)
        for i in range(NCH):
            loA, hiA, loB, hiB = dq[i]
            eloA, ehiA, eloB, ehiB = eq[i]
            xp = np.concatenate(
                [loA, eloA, loB, eloB, hiA, ehiA, hiB, ehiB],
                axis=1)
            im[f"xp{i}"] = xp.astype(_BF_NP)

        rv = np.zeros(128 * RW, np.float32)
        rv[:len(dense_r_sp[c])] = dense_r_sp[c]
        im["xr"] = rv.reshape(128, RW).astype(_F8_NP)

        aux = np.zeros((128, AUXW), np.float32)
        aux[:, REW:REW + FS] = 0.5
        rev = edge_r_sp[c]
        rem = np.zeros(128 * REW, np.float32)
        rem[:len(rev)] = rev
        aux[:, 0:REW] = rem.reshape(128, REW)
        lay = []
        off = 0
        for b in bins[c]:
            n = int(n_list[b])
            if valid[b] is None:
                pb = p_full[b, :n, :n]
                ab = a_full[b, :n, :n]
            else:
                ix = np.ix_(valid[b], valid[b])
                pb = p_full[b][ix]
                ab = a_full[b][ix]
            aux[off:off + n, REW + off:REW + off + n] = pb
            aux[off:off + n, REW + FS + off:REW + FS + off + n] = ab
            aux[off:off + n, REW + 2 * FS + off:REW + 2 * FS + off + n] = 1.0
            lay.append((b, off, n))
            off += n
        small_layout.append(lay)
        im["aux"] = aux.astype(_BF_NP)

        crd = np.zeros((128, 2 * CW), np.float32)
        v = np.zeros(128 * CW, np.float32)
        v[:len(pc_sp[c])] = pc_sp[c]
        crd[:, 0:CW] = v.reshape(128, CW)
        v = np.zeros(128 * CW, np.float32)
        v[:len(pt_sp[c])] = pt_sp[c]
        crd[:, CW:2 * CW] = v.reshape(128, CW)
        im["crd"] = crd

        im["idn"] = ident
        in_maps.append(im)

    res = run_bass_kernel_spmd(nc, in_maps, core_ids=list(range(N_CORES)))
    if _want_results is not None:
        _want_results.append(res)

    # ---- host finalization (float64) ----
    st = [res.results[c]["st"].astype(np.float64) for c in range(N_CORES)]
    n_arr = n_list.astype(np.float64)
    cnt_coord = max(float(n_arr.sum()) * C, 1.0)
    cnt2 = max(float((n_arr ** 2).sum()), 1.0)
    LNH = float(np.log(0.5))

    s_dense = sum(float(s[:, 0].sum() + s[:, 1].sum()) for s in st)
    s_edge = sum(float(s[:, 2].sum() + s[:, 3].sum()) for s in st)
    # dense pad correction: each pad element contributes ln(.5) to both
    # chains -> (1 + 1/19)*ln(.5) per pad in the combined accumulator
    npad_dense = 4.0 * DENSE_QUADS * N_CORES - len(dense_p)
    s_dense -= npad_dense * (20.0 / 19.0) * LNH
    edge_sum = 0.95 * s_dense + 0.9 * s_edge
    edge_loss = -edge_sum / cnt2

    s_r2 = sum(float(s[:, 4].sum()) for s in st)
    s_re = sum(float(s[:, 5].sum()) for s in st)
    similarity_loss = (s_r2 - 2.0 * s_re + e_tot) / cnt2

    s_d2 = sum(float(s[:, 6].sum()) for s in st)
    s_h2 = sum(float(s[:, 7].sum()) for s in st)
    coord_mse = s_d2 / cnt_coord
    coord_smooth = 0.5 * (s_d2 - s_h2) / cnt_coord
    coord_loss = 0.7 * coord_mse + 0.3 * coord_smooth

    ari_loss = 0.0
    conf_pen = 0.0
    for c in range(N_CORES):
        for b, off, n in small_layout[c]:
            rows = st[c][off:off + n]
            nf = float(n)
            s_l1p = float(rows[:, 8].sum())
            s_pd = float(rows[:, 9].sum())
            s_pa = float(rows[:, 10].sum())
            s_p2 = float(rows[:, 11].sum()) - nf * (FS - nf) * 0.25
            s_abs = float(rows[:, 12].sum())
            if not (5.0 < nf <= 50.0):
                continue
            na = np.sqrt(max(s_p2, 0.0))
            nt = np.sqrt(max(float(ec_list[b]), 0.0))
            cos = s_pa / (max(na, EPS) * max(nt, EPS))
            n2 = max(nf * nf, 1.0)
            ent = -(s_l1p + s_pd) / n2
            contrast = s_abs / n2
            ari_loss += -cos - 0.2 * contrast
            conf_pen += ent

    dc = np.asarray(node_counts, np.float64) - n_arr
    count_loss = float(_huber(dc).mean())
    temp_reg = abs(float(temperature) - 1.0)
    res_reg = abs(float(residual_weight) - 0.5)

    total = (1.0 * coord_loss + 2.0 * edge_loss + 0.1 * count_loss
             + 0.3 * similarity_loss + 0.01 * (temp_reg + res_reg)
             + 1.0 * (ari_loss + 0.1 * conf_pen))
    return np.asarray(total, dtype=np.float32)
